# revision 1
# baseline (speedup 1.0000x reference)
"""Trainium2 Bass kernel for nn_BinConv2d: BN(train-mode) -> sign -> 3x3 conv.

Two launches on 8 cores, batch-sharded (2 images/core, 128 partitions =
2 img x 64 ch):

  Launch A (stats), engine-split so neither engine is the wall: DVE
    bn_stats takes 66 of the 98 512-elem groups, ACT computes sum(x) /
    sum(x^2) for the other 32 via Copy/Square with accum_out (per-1024
    sub-groups to bound f32 accumulation error).  Host combines both
    shares in f64, pools across cores, and folds BN+sign into one
    per-channel threshold t_c = mean_c - bias_c*sqrt(var_c+eps)/w_c.

  Launch B (conv), full 128x128 PE array per matmul: per image, a band
    strip holds sign(x) in fp8e4 ({-1,0,1} exact) with partitions =
    64 ch x 2 halves, the second half being the whole strip shifted up
    one row-slot (one contiguous SBUF->SBUF DMA per band, issued from
    the scalar queue right after its ACT).  An AP strip[:, 2k*226+dx]
    yields row 2k on one half and 2k+1 on the other, so one matmul
    contracts 128 partitions (2 rows x 64 ch; fp16 weights x fp8 strip)
    and yields 128 psum partitions (2 output rows x 64 oc) over 226
    cols; 6 matmuls accumulate a 2-row tile.  Matmuls run weight-outer
    over 7 one-per-bank psum tiles per half-band (weights switch 6x per
    half-band; two accumulation groups must never share a PSUM bank).
    x bands prefetch one band ahead on the sync queue; y is written by
    gpsimd in the parity-split device layout [128, 2, 112, 224]
    (contiguous 12.5KB/partition descriptors) and unshuffled on host.

  Measured: stats ~83-90us + conv ~187-195us = ~277us HW (best; mean
  ~280, device variance ~+/-10us), rel err 2.0e-4 (fp16 weights; exact
  f32 sign) vs 318us baseline.  Binarize + shift-copies run in three
  chunks per band; matmuls run in quarter-band chunks (4+3+4+3 tiles)
  so two chunks share the 8 psum banks and evacuations overlap the next
  chunk's matmuls; stats x-pool is 5-deep so DMA runs ahead of the
  slower ACT-share chunks.
"""

import sys

if "/opt/trn_rl_repo" not in sys.path:
    sys.path.insert(0, "/opt/trn_rl_repo")

import numpy as np

import concourse.bacc as bacc
import concourse.tile as tile
from concourse import mybir
from concourse.bass_utils import run_bass_kernel_spmd

F32 = mybir.dt.float32
F16 = mybir.dt.float16
F8 = mybir.dt.float8e4

N_CORES = 8
N, C, H, W = 16, 64, 224, 224
BN_EPS = 1e-4
BAND = 28              # output rows per band
NB = H // BAND         # 8 bands
WP = W + 2             # padded strip width (226)
NT = BAND // 2         # 14 tiles per band, 2 output rows each
SLOTS = BAND + 2       # 30 strip slots per band
STRIP_LEN = SLOTS * WP
HH = H // 2            # 112


ACT_CHUNKS = {3, 6, 9}   # stats chunks routed to the scalar engine
N_DVE_GROUPS = 66        # the rest go through DVE bn_stats
NAC = 17                 # ACT accumulation slots (1024-elem sub-groups)


def build_stats_nc(repeat=1):
    """Per-core moments of x_s [128, 50176] f32, split across engines:
    DVE bn_stats for 66 of the 98 512-elem groups -> stats [128, 2]
    (mean, var over the DVE share); ACT computes per-1024-elem sums of x
    and x^2 via accum_out for the other 32 groups -> asum/asq [128, 17].
    The host combines both shares in f64."""
    nc = bacc.Bacc()
    cols = H * W
    x_s = nc.declare_dram_parameter("x_s", [128, cols], F32, isOutput=False)
    stats_out = nc.declare_dram_parameter("stats", [128, 2], F32, isOutput=True)
    asum_out = nc.declare_dram_parameter("asum", [128, NAC], F32, isOutput=True)
    asq_out = nc.declare_dram_parameter("asq", [128, NAC], F32, isOutput=True)

    n_groups = cols // 512  # 98
    chunk_groups = [2, 4, 8]
    g = n_groups - sum(chunk_groups)
    while g > 0:
        chunk_groups.append(min(13, g))
        g -= min(13, g)
    base = 12  # tile sized for the largest (13-group) chunk

    with tile.TileContext(nc) as tc:
        with (
            tc.tile_pool(name="xc", bufs=5) as xpool,
            tc.tile_pool(name="acc", bufs=1) as apool,
        ):
            stats = apool.tile([128, N_DVE_GROUPS, 6], F32)
            mv = apool.tile([128, 2], F32)
            asum = apool.tile([128, NAC], F32)
            asq = apool.tile([128, NAC], F32)
            trash = apool.tile([128, 1024], F16)

            def emit_all():
                g0 = 0
                di = 0
                ai = 0
                for ci, ng in enumerate(chunk_groups):
                    xt = xpool.tile([128, (base + 1) * 512], F32, tag="xt")
                    nc.sync.dma_start(
                        out=xt[:, : ng * 512],
                        in_=x_s[:, g0 * 512 : (g0 + ng) * 512],
                    )
                    if ci in ACT_CHUNKS:
                        off = 0
                        rem = ng * 512
                        while rem > 0:
                            sz = min(1024, rem)
                            nc.scalar.activation(
                                out=trash[:, 0:sz],
                                in_=xt[:, off : off + sz],
                                func=mybir.ActivationFunctionType.Copy,
                                accum_out=asum[:, ai : ai + 1],
                            )
                            nc.scalar.activation(
                                out=trash[:, 0:sz],
                                in_=xt[:, off : off + sz],
                                func=mybir.ActivationFunctionType.Square,
                                accum_out=asq[:, ai : ai + 1],
                            )
                            off += sz
                            rem -= sz
                            ai += 1
                    else:
                        for g in range(ng):
                            nc.vector.bn_stats(
                                out=stats[:, di, :],
                                in_=xt[:, g * 512 : (g + 1) * 512],
                            )
                            di += 1
                    g0 += ng
                assert ai == NAC and di == N_DVE_GROUPS
                nc.vector.bn_aggr(out=mv[:], in_=stats[:])
                nc.sync.dma_start(out=stats_out[:], in_=mv[:])
                nc.sync.dma_start(out=asum_out[:], in_=asum[:])
                nc.sync.dma_start(out=asq_out[:], in_=asq[:])

            if repeat == 1:
                emit_all()
            else:
                with tc.For_i(0, repeat, 1):
                    emit_all()
    nc.compile()
    return nc


def build_conv_nc(repeat=1):
    """Per-core conv kernel: x_b [128, 224, 224] f32 (2 img x 64 ch),
    wts [128, 12, 128] fp16 lhsT bank, tneg [128,1], cbias [128,1]
    -> y [128, 2, 112, 224] f32 (parity-split device layout)."""
    nc = bacc.Bacc()
    x_b = nc.declare_dram_parameter("x_b", [128, H, W], F32, isOutput=False)
    wts = nc.declare_dram_parameter("wts", [128, 12, 128], F16, isOutput=False)
    tneg = nc.declare_dram_parameter("tneg", [128, 1], F32, isOutput=False)
    cbias = nc.declare_dram_parameter("cbias", [128, 1], F32, isOutput=False)
    y = nc.declare_dram_parameter("y", [128, 2, HH, W], F32, isOutput=True)

    with tile.TileContext(nc) as tc:
        with (
            tc.tile_pool(name="const", bufs=1) as cpool,
            tc.tile_pool(name="xband", bufs=3) as xpool,
            tc.tile_pool(name="stage", bufs=2) as opool,
            tc.tile_pool(name="psum", bufs=8, space="PSUM") as ppool,
        ):
            wsb = cpool.tile([128, 12, 128], F16)
            nc.sync.dma_start(out=wsb[:], in_=wts[:])
            tsb = cpool.tile([128, 1], F32)
            nc.sync.dma_start(out=tsb[:], in_=tneg[:])
            bsb = cpool.tile([128, 1], F32)
            nc.sync.dma_start(out=bsb[:], in_=cbias[:])

            strips = [
                [
                    cpool.tile([128, STRIP_LEN], F8, name=f"strip{im}_{pb}",
                               tag=f"strip{im}_{pb}")
                    for pb in range(2)
                ]
                for im in range(2)
            ]
            for im in range(2):
                for pb in range(2):
                    s3 = strips[im][pb].rearrange("p (s c) -> p s c", c=WP)
                    nc.vector.memset(s3[:, :, 0], 0.0)
                    nc.vector.memset(s3[:, :, WP - 1], 0.0)
                    nc.vector.memset(s3[:, 0, :], 0.0)
                    nc.vector.memset(s3[:, SLOTS - 1, :], 0.0)

            def load_band(b):
                r0 = b * BAND
                lo_r = max(r0 - 1, 0)
                hi_r = min(r0 + BAND + 1, H)
                s0 = lo_r - (r0 - 1)
                nr = hi_r - lo_r
                xt = xpool.tile([128, SLOTS, W], F32, tag="xt")
                if b == 0:
                    # split the first load so ACT chunk 1 (slots <10)
                    # starts earlier
                    nc.sync.dma_start(out=xt[:, s0:10, :],
                                      in_=x_b[:, lo_r : lo_r + 10 - s0, :])
                    nc.sync.dma_start(out=xt[:, 10 : s0 + nr, :],
                                      in_=x_b[:, lo_r + 10 - s0 : hi_r, :])
                else:
                    nc.sync.dma_start(out=xt[:, s0 : s0 + nr, :],
                                      in_=x_b[:, lo_r:hi_r, :])
                return xt, s0, nr

            def emit_all():
                xts = {0: load_band(0)}
                for b in range(NB):
                    if b + 1 < NB:
                        xts[b + 1] = load_band(b + 1)
                    xt, s0, nr = xts.pop(b)
                    r0 = b * BAND

                    sA = strips[0][b % 2]
                    sB = strips[1][b % 2]
                    s3A = sA.rearrange("p (s c) -> p s c", c=WP)
                    s3B = sB.rearrange("p (s c) -> p s c", c=WP)

                    if b == NB - 1:
                        # bottom pad slot (stale data from band NB-3)
                        nc.vector.memset(s3A[0:64, SLOTS - 1, :], 0.0)
                        nc.vector.memset(s3B[64:128, SLOTS - 1, :], 0.0)

                    # binarize in half-band chunks, each followed by its
                    # one-slot-up copy, so the first matmuls start after
                    # ~1/4 of the band's ACT work instead of all of it.
                    # chunk 1 covers strip slots s0..16, chunk 2 17..s0+nr.
                    # (ACT slot range, copy hi-end): the last copy
                    # extends to slot 30 so hi slot 28 gets lo slot 29
                    # (real halo row, or the zeroed pad on the last band).
                    chunks = (((s0, 10), 10), ((10, 17), 17),
                              ((17, s0 + nr), SLOTS))
                    for (lo_h, hi_h), ce in chunks:
                        nc.scalar.activation(
                            out=s3A[0:64, lo_h:hi_h, 1 : 1 + W],
                            in_=xt[0:64, lo_h:hi_h, :],
                            func=mybir.ActivationFunctionType.Sign,
                            bias=tsb[0:64],
                        )
                        c0 = max(lo_h - 1, 0) * WP
                        nc.scalar.dma_start(
                            out=sA[64:128, c0 : (ce - 1) * WP],
                            in_=sA[0:64, c0 + WP : ce * WP],
                        )
                    for (lo_h, hi_h), ce in chunks:
                        nc.scalar.activation(
                            out=s3B[64:128, lo_h:hi_h, 1 : 1 + W],
                            in_=xt[64:128, lo_h:hi_h, :],
                            func=mybir.ActivationFunctionType.Sign,
                            bias=tsb[64:128],
                        )
                        c0 = max(lo_h - 1, 0) * WP
                        nc.scalar.dma_start(
                            out=sB[0:64, c0 : (ce - 1) * WP],
                            in_=sB[64:128, c0 + WP : ce * WP],
                        )

                    stgs = {}
                    QT = ((0, 4), (4, 7), (7, 11), (11, 14))

                    def do_chunk(im, q):
                        strip = sA if im == 0 else sB
                        if im not in stgs:
                            stgs[im] = opool.tile([128, NT, W], F32,
                                                  tag=f"stg{im}",
                                                  name=f"stg{b}_{im}")
                        stg = stgs[im]
                        ta, tb = QT[q]
                        # weight-outer per quarter-band: 3-4 live psum
                        # tiles, so two chunks fit in the 8 banks and the
                        # next chunk's matmuls overlap this one's evacs.
                        pss = [
                            ppool.tile([128, WP], F32, tag="ps",
                                       name=f"ps{b}_{im}_{t}")
                            for t in range(ta, tb)
                        ]
                        for m in range(6):
                            pair, dx = divmod(m, 3)
                            for j, t in enumerate(range(ta, tb)):
                                st = (2 * t + 2 * pair) * WP + dx
                                nc.tensor.matmul(
                                    pss[j][:, 0:WP],
                                    wsb[:, im * 6 + m, :],
                                    strip[:, st : st + WP],
                                    start=(m == 0),
                                    stop=(m == 5),
                                )
                        for j, t in enumerate(range(ta, tb)):
                            nc.vector.tensor_scalar(
                                out=stg[:, t, :],
                                in0=pss[j][:, 0:W],
                                scalar1=bsb[:],
                                scalar2=None,
                                op0=mybir.AluOpType.add,
                            )
                        if q in (1, 3):
                            h0 = 0 if q == 1 else NT // 2
                            nc.gpsimd.dma_start(
                                out=y[:, im,
                                      r0 // 2 + h0 : r0 // 2 + h0 + NT // 2, :],
                                in_=stg[:, h0 : h0 + NT // 2, :],
                            )

                    if b == NB - 1:
                        order = [(im, q) for q in range(4) for im in (0, 1)]
                    else:
                        order = [(im, q) for im in (0, 1) for q in range(4)]
                    for im, q in order:
                        do_chunk(im, q)

            if repeat == 1:
                emit_all()
            else:
                with tc.For_i(0, repeat, 1):
                    emit_all()
    nc.compile()
    return nc


_cache = {}


def _get(name, builder):
    if name not in _cache:
        _cache[name] = builder()
    return _cache[name]


def _prep_conv_inputs(x, bn_weight, bn_bias, conv_weight, conv_bias, stats):
    # per-core results: DVE share (mean, var over 66*512 elems) + ACT
    # share (17 sums of x and x^2) -> exact per-(img,ch) moments in f64,
    # then pool to per-channel batch stats
    ipc = N // N_CORES
    cols = H * W
    n_dve = N_DVE_GROUPS * 512
    meanM = np.empty((N, C), np.float64)
    varM = np.empty((N, C), np.float64)
    for c in range(N_CORES):
        r = stats[c]
        mv = r["stats"].astype(np.float64)
        tot = mv[:, 0] * n_dve + r["asum"].astype(np.float64).sum(1)
        tot2 = (mv[:, 1] + mv[:, 0] ** 2) * n_dve + r["asq"].astype(
            np.float64
        ).sum(1)
        mean_p = tot / cols
        var_p = tot2 / cols - mean_p**2
        s = np.stack([mean_p, var_p], -1).reshape(ipc, C, 2)
        meanM[ipc * c : ipc * (c + 1)] = s[..., 0]
        varM[ipc * c : ipc * (c + 1)] = s[..., 1]
    m = meanM.mean(axis=0)
    v = (varM + meanM**2).mean(axis=0) - m**2
    t = m - bn_bias.astype(np.float64) * np.sqrt(v + BN_EPS) / bn_weight.astype(
        np.float64
    )
    tneg = np.tile((-t).astype(np.float32), 2)[:, None]  # [128,1]
    cb = np.tile(conv_bias.astype(np.float32), 2)[:, None]

    # lhsT bank [128, 12, 128]: m = img*6 + pair*3 + dx.
    wts = np.zeros((128, 12, 128), np.float32)
    for im in range(2):
        for pair in range(2):
            for dx in range(3):
                mi = im * 6 + pair * 3 + dx
                for h in range(2):
                    a_slot = h if im == 0 else 1 - h
                    for bcol in range(2):
                        dy = a_slot - bcol + 2 * pair
                        if 0 <= dy <= 2:
                            wts[
                                h * 64 : h * 64 + 64,
                                mi,
                                bcol * 64 : bcol * 64 + 64,
                            ] = conv_weight[:, :, dy, dx].T
    return wts.astype(np.float16), tneg, cb


def _unshuffle_y(arr, ipc):
    # arr [128, 2, 112, 224]: [b*64+oc, im, r2, col] -> [im, oc, 2*r2+b, col]
    a = arr.reshape(2, C, 2, HH, W)            # [b, oc, im, r2, col]
    a = a.transpose(2, 1, 3, 0, 4)             # [im, oc, r2, b, col]
    return a.reshape(ipc, C, H, W)


def kernel(x, bn_weight, bn_bias, conv_weight, conv_bias):
    x = np.ascontiguousarray(np.asarray(x), dtype=np.float32)
    bn_weight = np.asarray(bn_weight, dtype=np.float32)
    bn_bias = np.asarray(bn_bias, dtype=np.float32)
    conv_weight = np.asarray(conv_weight, dtype=np.float32)
    conv_bias = np.asarray(conv_bias, dtype=np.float32)

    ipc = N // N_CORES
    nc_s = _get("stats", build_stats_nc)
    in_maps = [
        {"x_s": x[ipc * c : ipc * (c + 1)].reshape(128, H * W)}
        for c in range(N_CORES)
    ]
    res = run_bass_kernel_spmd(nc_s, in_maps, list(range(N_CORES))).results
    stats = [res[c] for c in range(N_CORES)]

    wts, tneg, cb = _prep_conv_inputs(
        x, bn_weight, bn_bias, conv_weight, conv_bias, stats
    )

    nc_c = _get("conv", build_conv_nc)
    in_maps = [
        {
            "x_b": x[ipc * c : ipc * (c + 1)].reshape(128, H, W),
            "wts": wts,
            "tneg": tneg,
            "cbias": cb,
        }
        for c in range(N_CORES)
    ]
    res = run_bass_kernel_spmd(nc_c, in_maps, list(range(N_CORES))).results
    y = np.concatenate(
        [_unshuffle_y(res[c]["y"], ipc) for c in range(N_CORES)], axis=0
    )
    return y

